# revision 2
# baseline (speedup 1.0000x reference)
"""ComplexLSTM Trainium2 kernel.

Problem: x [2, 64, 128, 1024] (real/imag, B, I, T) -> out [2, 64, 256, 1024].
Four real LSTM applications combined as L_r = r(xr) - i(xim), L_i = r(xim) + i(xr).

Sharding: 2 weight-sets x 128 sequences = 256 independent sequences;
8 cores x 32 sequences (cores 0-3: r-weights, cores 4-7: i-weights).

v2 changes vs baseline:
- ACT engine uses ONLY Sigmoid (single activation table, no table swaps):
    tanh(x) = 2*sigmoid(2x) - 1 folded via host-side weight scaling.
  Gate blocks in torch order [i,i,f,f,g,g,o,o]; g rows of Wih/Whh/bias
  pre-scaled x2; ALL Whh rows pre-scaled x2 because h is stored halved
  (h' = h/2 falls out of the one-op STT fixup); host output combine does x2.
- One sigmoid ACT op per step over all 8 gate blocks; per-step elementwise:
    s   = sigmoid(gates)                       ACT
    v   = (s_g - 0.5) * s_i                    DVE  (= ig/2)
    u   = s_f * c_prev                         DVE
    c   = 2*v + u                              DVE
    tch = sigmoid(2c)                          ACT  (= (tanh(c)+1)/2)
    h'  = (tch - 0.5) * s_o                    DVE  (= h/2, bf16 into HIST)
- Recurrent MMs ordered k-major; PSUM 4-deep so PE runs ahead on next-step
  bias/x-projection while the elementwise chain completes.
- HIST chunks DMA'd transposed so DRAM hist is [batch, H, T]: host combine
  is pure elementwise (no transposes).
"""

import numpy as np
import ml_dtypes
from contextlib import ExitStack

import concourse.bass as bass
import concourse.bacc as bacc
import concourse.tile as tile
from concourse import mybir
from concourse.bass_utils import run_bass_kernel_spmd

BF16 = mybir.dt.bfloat16
F32 = mybir.dt.float32
AF = mybir.ActivationFunctionType
OP = mybir.AluOpType

B, I, T_FULL, H = 64, 128, 1024, 256
NB = 32          # sequences per core
NCORES = 8
TC = 128         # history chunk (steps per output DMA)
XC = 128         # x input chunk (steps per input DMA)

_cache = {}


def build(T):
    nc = bacc.Bacc("TRN2", target_bir_lowering=False, debug=False)

    tc_hist = max(1, min(TC, T))
    xc = max(1, min(XC, T))
    assert T % tc_hist == 0 and T % xc == 0

    xT_d = nc.declare_dram_parameter("xT", [128, T, NB], BF16, isOutput=False)
    whhT_d = nc.declare_dram_parameter("whhT", [128, 2, 8, 128], BF16, isOutput=False)
    wihT_d = nc.declare_dram_parameter("wihT", [128, 8, 128], BF16, isOutput=False)
    biasK_d = nc.declare_dram_parameter("biasK", [8, 128], BF16, isOutput=False)
    ind_d = nc.declare_dram_parameter("ind", [8, 8 * NB], BF16, isOutput=False)
    hist_d = nc.declare_dram_parameter("hist", [NB, 2 * 128, T], BF16, isOutput=True)

    with tile.TileContext(nc) as tc, ExitStack() as ctx:
        consts = ctx.enter_context(tc.tile_pool(name="consts", bufs=1))
        xin = ctx.enter_context(tc.tile_pool(name="xin", bufs=2))
        hpool = ctx.enter_context(tc.tile_pool(name="hist", bufs=2))
        psum = ctx.enter_context(tc.tile_pool(name="psum", bufs=4, space="PSUM"))
        sml = ctx.enter_context(tc.tile_pool(name="small", bufs=3))
        cpool = ctx.enter_context(tc.tile_pool(name="cpool", bufs=3))

        WHH = consts.tile([128, 2, 8, 128], BF16)
        nc.sync.dma_start(WHH[:], whhT_d[:])
        WIH = consts.tile([128, 8, 128], BF16)
        nc.sync.dma_start(WIH[:], wihT_d[:])
        BIASK = consts.tile([8, 128], BF16)
        nc.sync.dma_start(BIASK[:], biasK_d[:])
        IND = consts.tile([8, 8 * NB], BF16)
        nc.sync.dma_start(IND[:], ind_d[:])

        XBUF = None
        HIST = None
        c_prev = None
        h_prev = None  # AP into HIST for h'(t-1), [128, 2, NB]

        for t in range(T):
            tl = t % xc
            if tl == 0:
                XBUF = xin.tile([128, xc, NB], BF16, tag="xbuf")
                nc.sync.dma_start(XBUF[:], xT_d[:, t:t + xc, :])
            th = t % tc_hist
            if th == 0:
                HIST = hpool.tile([128, 2, NB, tc_hist], BF16, tag="hist")

            g_ps = psum.tile([128, 8, NB], F32, tag="gates")
            # bias via indicator matmul (start=True clears PSUM), then x-proj,
            # then the recurrent part (k-major so h chunks are consumed asap)
            nc.tensor.matmul(g_ps[:], BIASK[:], IND[:], start=True, stop=False)
            for m in range(8):
                nc.tensor.matmul(
                    g_ps[:, m, :], WIH[:, m, :], XBUF[:, tl, :],
                    start=False, stop=(t == 0 and m == 7),
                )
            if t > 0:
                for k in range(2):
                    for m in range(8):
                        nc.tensor.matmul(
                            g_ps[:, m, :], WHH[:, k, m, :], h_prev[:, k, :],
                            start=False, stop=(k == 1),
                        )

            # s = sigmoid over the gate blocks [i,i,f,f,g,g,o,o]; split so the
            # chain-critical [i,f,g] sigmoid issues first, o's overlaps DVE work
            s = sml.tile([128, 8, NB], F32, tag="s")
            nc.scalar.activation(s[:, 0:6, :], g_ps[:, 0:6, :], AF.Sigmoid)
            nc.scalar.activation(s[:, 6:8, :], g_ps[:, 6:8, :], AF.Sigmoid)

            # v = (s_g - 0.5) * s_i   (= ig/2)
            v = sml.tile([128, 2, NB], F32, tag="v")
            nc.vector.scalar_tensor_tensor(
                v[:], s[:, 4:6, :], 0.5, s[:, 0:2, :], OP.subtract, OP.mult)
            c_new = cpool.tile([128, 2, NB], F32, tag="c")
            if t > 0:
                u = sml.tile([128, 2, NB], F32, tag="u")
                nc.vector.tensor_tensor(u[:], s[:, 2:4, :], c_prev[:], OP.mult)
                nc.vector.scalar_tensor_tensor(
                    c_new[:], v[:], 2.0, u[:], OP.mult, OP.add)
            else:
                nc.vector.tensor_scalar_mul(c_new[:], v[:], 2.0)
            # tch = sigmoid(2c) = (tanh(c)+1)/2
            tch = sml.tile([128, 2, NB], F32, tag="tch")
            nc.scalar.activation(tch[:], c_new[:], AF.Sigmoid, scale=2.0)
            # h' = (tch - 0.5) * s_o  (= h/2), bf16 straight into the history
            h_slot = HIST[:, :, :, th]
            nc.vector.scalar_tensor_tensor(
                h_slot, tch[:], 0.5, s[:, 6:8, :], OP.subtract, OP.mult)

            c_prev = c_new
            h_prev = HIST[:, :, :, th]

            if th == tc_hist - 1:
                t0 = t - (tc_hist - 1)
                for k in range(2):
                    nc.sync.dma_start(
                        hist_d[:, 128 * k:128 * (k + 1), t0:t0 + tc_hist]
                        .transpose([1, 0, 2]),
                        HIST[:, k, :, :],
                    )
    nc.compile()
    return nc


def _get_nc(T):
    if T not in _cache:
        _cache[T] = build(T)
    return _cache[T]


def _prep_core_inputs(x, Wih, Whh, bih, bhh, T):
    """Per weight-set host prep (torch gate order i,f,g,o kept as-is).

    Scaling for the sigmoid-only device kernel:
      g rows (512:768) of Wih/Whh/bias x2    (tanh(x) = 2*sigmoid(2x)-1)
      ALL Whh rows x2                        (h stored halved on device)
    """
    Wihp = np.asarray(Wih, np.float32).copy()
    Whhp = np.asarray(Whh, np.float32).copy()
    biasp = (np.asarray(bih, np.float32) + np.asarray(bhh, np.float32)).copy()
    Wihp[512:768] *= 2.0
    biasp[512:768] *= 2.0
    Whhp[512:768] *= 2.0
    Whhp *= 2.0

    whhT = Whhp.reshape(8, 128, 2, 128).transpose(3, 2, 0, 1)  # [p,k,m,j]
    wihT = Wihp.reshape(8, 128, 128).transpose(2, 0, 1)        # [p,m,j]
    biasK = biasp.reshape(8, 128)
    whhT = np.ascontiguousarray(whhT).astype(ml_dtypes.bfloat16)
    wihT = np.ascontiguousarray(wihT).astype(ml_dtypes.bfloat16)
    biasK = biasK.astype(ml_dtypes.bfloat16)

    # batch-128 for this weight set: seqs 0-63 = x_real, 64-127 = x_imag
    xTs = []
    xall = np.concatenate([np.asarray(x)[0], np.asarray(x)[1]], axis=0)  # [128, I, T]
    for g in range(4):
        sl = xall[32 * g:32 * g + 32]             # [32, I, T]
        xT = sl.transpose(1, 2, 0)[:, :T, :]      # [I, T, 32]
        xTs.append(np.ascontiguousarray(xT).astype(ml_dtypes.bfloat16))
    return whhT, wihT, biasK, xTs


def _bf16_to_f32(a):
    return (a.view(np.uint16).astype(np.uint32) << 16).view(np.float32)


def _run(x, Wih_r, Whh_r, bih_r, bhh_r, Wih_i, Whh_i, bih_i, bhh_i, T,
         trace=False, tmpdir=None):
    nc = _get_nc(T)
    ind = np.kron(np.eye(8), np.ones((1, NB))).astype(ml_dtypes.bfloat16)

    whhT_r, wihT_r, biasK_r, xTs_r = _prep_core_inputs(x, Wih_r, Whh_r, bih_r, bhh_r, T)
    whhT_i, wihT_i, biasK_i, xTs_i = _prep_core_inputs(x, Wih_i, Whh_i, bih_i, bhh_i, T)

    in_maps = []
    for core in range(NCORES):
        ws = core // 4
        g = core % 4
        whhT, wihT, biasK = (whhT_r, wihT_r, biasK_r) if ws == 0 else (whhT_i, wihT_i, biasK_i)
        xT = (xTs_r if ws == 0 else xTs_i)[g]
        in_maps.append({
            "xT": xT, "whhT": whhT, "wihT": wihT, "biasK": biasK, "ind": ind,
        })
    res = run_bass_kernel_spmd(nc, in_maps, core_ids=list(range(NCORES)),
                               trace=trace, tmpdir=tmpdir)
    results = res.results

    # hist per core: [32, 256, T] bf16 holding h/2.  out = 2*(r +- i).
    out = np.empty((2, B, H, T), np.float32)
    for a in range(2):
        r1 = _bf16_to_f32(results[a]["hist"])       # r(x_real), seqs 32a..
        i2 = _bf16_to_f32(results[6 + a]["hist"])   # i(x_imag)
        np.subtract(r1, i2, out=out[0, 32 * a:32 * a + 32])
        r2 = _bf16_to_f32(results[2 + a]["hist"])   # r(x_imag)
        i1 = _bf16_to_f32(results[4 + a]["hist"])   # i(x_real)
        np.add(r2, i1, out=out[1, 32 * a:32 * a + 32])
    out *= 2.0
    return out, res


def kernel(x, Wih_r, Whh_r, bih_r, bhh_r, Wih_i, Whh_i, bih_i, bhh_i):
    out, _ = _run(x, Wih_r, Whh_r, bih_r, bhh_r,
                  Wih_i, Whh_i, bih_i, bhh_i, T_FULL)
    return out
